# revision 55
# baseline (speedup 1.0000x reference)
"""TP=8 LSTM decoder kernel for trn2 (Bass, raw engine programming), v2.

Math (per reference, with feedback folded into the recurrence):
    x(t) = y(t-1) = h(t-1) @ W_out.T + b_out   (x(0) = 0)
    gates(t) = x(t) @ W_ih.T + h(t-1) @ W_hh.T + b
             = h(t-1) @ W_comb.T + b'          (t >= 1)
    where W_comb = W_hh + W_ih @ W_out,  b' = b_ih + b_hh + W_ih @ b_out.
Step 0 (x=0) is computed on the host; the device runs steps 1..T-1 and
computes y(t) = h(t) @ W_out.T + b_out for t = 0..T-1.

v2 vs v1:
  - all matmul operands fp16 (1 cycle/row on PE vs fp32's 4), fp32 PSUM
  - y output TP-sliced 8 ways (each core computes mel rows [64s, 64s+64));
    host assembles from all 8 cores' DRAM outputs
  - per-core gate rows split in two 128-row blocks (r0/r1); block r0's
    cell + transpose + broadcast overlap the PE phase for block r1, and
    block r1's tail overlaps the next step's r0 phase
  - h exchanged via SBUF-to-SBUF remote_dma_broadcast (16 KB per block)
    instead of an AllGather bounced through DRAM
  - receive semaphores split by step parity to close the 1-step-skew
    anonymous-counter race

Sharding: core s owns h rows [256s, 256s+256) as two blocks of 128
(gate col order [i|f|o|g] per block) and mel rows [64s, 64s+64).
"""

import numpy as np

B = 64          # batch
H = 2048        # lstm dim
MEL = 512
NC = 8
HS = H // NC    # 256 h rows per core
KCH = H // 128  # 16 contraction chunks
T = 512


def build_nc(n_steps: int, debug: bool = False):
    import concourse.bass as bass
    import concourse.bacc as bacc
    import concourse.mybir as mybir
    from concourse.bass import ts, ds

    f32 = mybir.dt.float32
    f16 = mybir.dt.float16
    Tn = n_steps

    nc = bacc.Bacc("TRN2", target_bir_lowering=False, debug=debug,
                   num_devices=NC)

    # ---------------- DRAM I/O ----------------
    w_d = nc.dram_tensor("w", [128, KCH * 1024], f16, kind="ExternalInput")
    wy_d = nc.dram_tensor("wy", [128, KCH * 64], f16, kind="ExternalInput")
    h0_d = nc.dram_tensor("h0", [128, KCH * 64], f16, kind="ExternalInput")
    c0_d = nc.dram_tensor("c0", [B, 2 * 128], f32, kind="ExternalInput")
    bps_d = nc.dram_tensor("bps", [1, 1024], f16, kind="ExternalInput")
    bout_d = nc.dram_tensor("bout", [1, 64], f16, kind="ExternalInput")
    ones_d = nc.dram_tensor("ones", [1, B], f16, kind="ExternalInput")
    id_d = nc.dram_tensor("ident", [B, B], f32, kind="ExternalInput")
    yout_d = nc.dram_tensor("yout", [Tn, B, 64], f32, kind="ExternalOutput")

    ctx_list = []

    def sb(name, shape, dt=f32):
        t = nc.sbuf_tensor(name, shape, dt)
        ctx_list.append(t)
        return t.__enter__()

    def ps(name, shape):
        t = nc.psum_tensor(name, shape, f32)
        ctx_list.append(t)
        return t.__enter__()

    def sem(name):
        t = nc.semaphore(name)
        ctx_list.append(t)
        return t.__enter__()

    # ---------------- SBUF ----------------
    s_w = sb("s_w", [128, KCH * 1024], f16)    # 4 MB: chunk c -> [r0 512|r1 512]
    s_wy = sb("s_wy", [128, KCH * 64], f16)    # 256 KB
    s_h = [sb("s_h0", [128, KCH * 64], f16),   # h.T chunks, dbl-buffered
           sb("s_h1", [128, KCH * 64], f16)]
    s_bps = sb("s_bps", [1, 1024], f16)
    s_bout = sb("s_bout", [1, 64], f16)
    s_ones = sb("s_ones", [1, B], f16)
    s_id = sb("s_id", [B, B])
    s_c = sb("s_c", [B, 2 * 256])              # c state, parity cols [r0|r1]
    s_act = sb("s_act", [B, 2 * 512])          # per block: sig(i|f|o) 384 | tg 128
    s_tc = sb("s_tc", [B, 2 * 128])            # tanh(c) per block
    s_t1a = sb("s_t1a", [B, 128])
    s_t2a = sb("s_t2a", [B, 128])
    s_t1b = sb("s_t1b", [B, 128])
    s_t2b = sb("s_t2b", [B, 128])
    s_hraw = sb("s_hraw", [B, 256])            # h fp32 [r0 128 | r1 128]
    s_stage = sb("s_stage", [128, 2 * 128], f16)  # h.T bcast staging, parity
    s_wu = sb("s_wu", [128, 64], f16)          # warm-up broadcast scratch
    s_y = sb("s_y", [B, 2 * 64])               # y slice staging, parity

    # ---------------- PSUM (one bank each) ----------------
    p_g0 = ps("p_g0", [B, 512])     # block r0 gates [i|f|o|g] x128
    p_g1 = ps("p_g1", [B, 512])     # block r1 gates
    p_y = ps("p_y", [B, 64])        # y slice [batch, mel-slice]
    p_tr0 = ps("p_tr0", [128, 64])  # h.T block r0
    p_tr1 = ps("p_tr1", [128, 64])  # h.T block r1
    p_dmy = ps("p_dmy", [B, 512])   # clock-keeper dummy target

    # ---------------- semaphores ----------------
    s_pre = sem("s_pre")        # preload DMAs (16 each)
    s_g0 = sem("s_g0")          # PE: p_g0 done (1/step)
    s_g1 = sem("s_g1")          # PE: p_g1 done (1/step)
    s_a0 = sem("s_a0")          # ACT: sig/tanh r0 done (1/step)
    s_a1 = sem("s_a1")          # ACT: sig/tanh r1 done (1/step)
    s_cd = sem("s_cd")          # DVE: c_new done (2/step)
    s_tcs = sem("s_tcs")        # ACT: tanh(c) done (2/step)
    s_hd = sem("s_hd")          # DVE: h fp32 done (2/step)
    s_tr = sem("s_tr")          # PE: transpose done (2/step)
    s_hl = sem("s_hl")          # DVE: stage copy into own h.T slot (2/step)
    s_hr0 = [sem("s_hr0e"), sem("s_hr0o")]   # remote r0 arrivals, by parity
    s_hr1 = [sem("s_hr1e"), sem("s_hr1o")]   # remote r1 arrivals, by parity
    s_send = sem("s_send")      # r0 bcast local completion, queue 0 (16/step)
    s_send1 = sem("s_send1")    # r1 bcast local completion, queue 1 (16/step)
    s_prep = sem("s_prep")      # broadcast descriptor-write completion
    s_ao0 = sem("s_ao0")        # ACT: sig(o) r0 done -> p_g0 reusable (1/step)
    s_ao1 = sem("s_ao1")        # ACT: sig(o) r1 done -> p_g1 reusable (1/step)
    s_warm = sem("s_warm")      # warm-up q0 completion
    s_warm1 = sem("s_warm1")    # warm-up q1 completion
    s_warmr = sem("s_warmr")    # warm-up arrivals (nobody waits)
    s_warmr1 = sem("s_warmr1")
    s_p2 = sem("s_p2")          # Pool: t2 = sig(f)*c_old done (2/step)
    s_yv = sem("s_yv")          # PE: y matmuls done (1/step)
    s_ycp = sem("s_ycp")        # DVE: y psum->sbuf copy done (1/step)
    s_ydma = sem("s_ydma")      # sync: y store done (16/step)

    N_PRE = 8
    mult = mybir.AluOpType.mult
    add = mybir.AluOpType.add
    Sig = mybir.ActivationFunctionType.Sigmoid
    Tanh = mybir.ActivationFunctionType.Tanh

    # number of steps t' in [1, t] with parity t'%2 == p
    def npar(t, p):
        if t <= 0:
            return 0
        return (t + 1) // 2 if p == 1 else t // 2

    with nc.Block() as block:

        # ------------- sync: preloads + y stores -------------
        @block.sync
        def _(sync):
            sync.dma_start(out=s_w[:, :], in_=w_d[:, :]).then_inc(s_pre, 16)
            sync.dma_start(out=s_wy[:, :], in_=wy_d[:, :]).then_inc(s_pre, 16)
            sync.dma_start(out=s_h[0][:, :], in_=h0_d[:, :]).then_inc(s_pre, 16)
            sync.dma_start(out=s_c[:, 0:256], in_=c0_d[:, :]).then_inc(s_pre, 16)
            sync.dma_start(out=s_bps[:, :], in_=bps_d[:, :]).then_inc(s_pre, 16)
            sync.dma_start(out=s_bout[:, :], in_=bout_d[:, :]).then_inc(s_pre, 16)
            sync.dma_start(out=s_ones[:, :], in_=ones_d[:, :]).then_inc(s_pre, 16)
            sync.dma_start(out=s_id[:, :], in_=id_d[:, :]).then_inc(s_pre, 16)
            for t in range(1, Tn + 1):
                sync.wait_ge(s_ycp, t)
                if t >= 2:
                    sync.wait_ge(s_ydma, 16 * (t - 1))
                sync.dma_start(
                    out=yout_d[t - 1, :, :],
                    in_=s_y[:, ts((t - 1) % 2, 64)],
                ).then_inc(s_ydma, 16)

        # ------------- PE -------------
        # transpose r0(t) is emitted between ev1 and od1 of step t;
        # transpose r1(t) is software-pipelined into step t+1 (after ev0),
        # so the PE never sits out the r1 cell-chain tail.
        @block.tensor
        def _(pe):
            pe.wait_ge(s_pre, 16 * N_PRE)
            for t in range(1, Tn + 1):
                hb = s_h[(t - 1) % 2]
                par = (t - 1) % 2
                if t <= Tn - 1:
                    # ---- bias + even chunks, bank r0 ----
                    if t >= 2:
                        pe.wait_ge(s_ao0, t - 1)         # p_g0 free
                        pe.wait_ge(s_hr0[par], 16 * npar(t - 1, par))
                    nc.tensor.matmul(p_g0[:, :], s_ones[:, :], s_bps[:, 0:512],
                                     start=True, stop=False)
                    for c in range(0, KCH, 2):           # even chunks (r0 of peers)
                        nc.tensor.matmul(p_g0[:, :], hb[:, ts(c, 64)],
                                         s_w[:, c * 1024: c * 1024 + 512],
                                         start=False, stop=False)
                if t <= Tn - 1:
                    # ---- odd chunks, bank r0 ----
                    if t >= 2:
                        pe.wait_ge(s_hr1[par], 16 * npar(t - 1, par))
                    for c in range(1, KCH, 2):           # odd chunks (r1 of peers)
                        mm = nc.tensor.matmul(p_g0[:, :], hb[:, ts(c, 64)],
                                              s_w[:, c * 1024: c * 1024 + 512],
                                              start=False, stop=(c == KCH - 1))
                    mm.then_inc(s_g0, 1)
                    # ---- bank r1: bias + evens ----
                    if t >= 2:
                        pe.wait_ge(s_ao1, t - 1)         # p_g1 free
                    nc.tensor.matmul(p_g1[:, :], s_ones[:, :], s_bps[:, 512:1024],
                                     start=True, stop=False)
                    for c in range(0, KCH, 2):
                        nc.tensor.matmul(p_g1[:, :], hb[:, ts(c, 64)],
                                         s_w[:, c * 1024 + 512: (c + 1) * 1024],
                                         start=False, stop=False)
                    # ---- bank r1: odds (first half) ----
                    for c in range(1, KCH // 2, 2):
                        nc.tensor.matmul(p_g1[:, :], hb[:, ts(c, 64)],
                                         s_w[:, c * 1024 + 512: (c + 1) * 1024],
                                         start=False, stop=False)
                    # transpose r0(t) mid-od1: the extra 4 matmuls of cover
                    # ahead of this wait absorb the r0 cell-chain latency
                    pe.wait_ge(s_hd, 2 * t - 1)
                    if t >= 2:
                        pe.wait_ge(s_hl, 2 * (t - 1) - 1)   # p_tr0 free
                    nc.tensor.transpose(p_tr0[:, :], s_hraw[:, 0:128],
                                        s_id[:, :]).then_inc(s_tr, 1)
                    # ---- bank r1: odds (second half) ----
                    for c in range(KCH // 2 + 1, KCH, 2):
                        mm = nc.tensor.matmul(p_g1[:, :], hb[:, ts(c, 64)],
                                              s_w[:, c * 1024 + 512: (c + 1) * 1024],
                                              start=False, stop=(c == KCH - 1))
                    mm.then_inc(s_g1, 1)
                # ---- y(t-1) slice ----
                pe.wait_ge(s_ycp, t - 1)                 # p_y free
                nc.tensor.matmul(p_y[:, :], s_ones[:, :], s_bout[:, :],
                                 start=True, stop=False)
                for c in range(KCH):
                    mm = nc.tensor.matmul(p_y[:, :], hb[:, ts(c, 64)],
                                          s_wy[:, ts(c, 64)],
                                          start=False, stop=(c == KCH - 1))
                mm.then_inc(s_yv, 1)
                if t <= Tn - 1:
                    # transpose r1 at iter end (sends as early as the cell
                    # chain allows; PE idles briefly but the exchange wins)
                    pe.wait_ge(s_hd, 2 * t)
                    if t >= 2:
                        pe.wait_ge(s_hl, 2 * (t - 1))    # p_tr1 free
                    nc.tensor.transpose(p_tr1[:, :], s_hraw[:, 128:256],
                                        s_id[:, :]).then_inc(s_tr, 1)

        # ------------- ACT -------------
        # per block: tanh(g) first, then sig(i|f) (releases the DVE c-chain),
        # then sig(o) (releases PSUM bank reuse), then tanh(c)
        @block.scalar
        def _(act):
            act.wait_ge(s_pre, 16 * N_PRE)
            for t in range(1, Tn):
                # block r0
                act.wait_ge(s_g0, t)
                if t >= 2:
                    act.wait_ge(s_hd, 2 * (t - 1) - 1)   # s_act r0 free
                nc.scalar.activation(s_act[:, 384:512], p_g0[:, 384:512], Tanh)
                nc.scalar.activation(s_act[:, 0:256], p_g0[:, 0:256], Sig)\
                    .then_inc(s_a0, 1)
                nc.scalar.activation(s_act[:, 256:384], p_g0[:, 256:384], Sig)\
                    .then_inc(s_ao0, 1)
                act.wait_ge(s_cd, 2 * t - 1)
                nc.scalar.activation(s_tc[:, 0:128], s_c[:, ts(t % 2, 256)][:, 0:128],
                                     Tanh).then_inc(s_tcs, 1)
                # block r1
                act.wait_ge(s_g1, t)
                if t >= 2:
                    act.wait_ge(s_hd, 2 * (t - 1))       # s_act r1 free
                nc.scalar.activation(s_act[:, 896:1024], p_g1[:, 384:512], Tanh)
                nc.scalar.activation(s_act[:, 512:768], p_g1[:, 0:256], Sig)\
                    .then_inc(s_a1, 1)
                nc.scalar.activation(s_act[:, 768:896], p_g1[:, 256:384], Sig)\
                    .then_inc(s_ao1, 1)
                act.wait_ge(s_cd, 2 * t)
                nc.scalar.activation(s_tc[:, 128:256],
                                     s_c[:, ts(t % 2, 256)][:, 128:256],
                                     Tanh).then_inc(s_tcs, 1)

        # ------------- DVE -------------
        @block.vector
        def _(dve):
            dve.wait_ge(s_pre, 16 * N_PRE)
            for t in range(1, Tn + 1):
                cold = s_c[:, ts((t - 1) % 2, 256)]
                cnew = s_c[:, ts(t % 2, 256)]
                if t <= Tn - 1:
                    stg = s_stage[:, ts(t % 2, 128)]
                    # ---- block r0: c, h, stage (bcast can start mid-step) ----
                    dve.wait_ge(s_a0, t)
                    nc.vector.scalar_tensor_tensor(
                        s_t1a[:, :], s_act[:, 0:128], 1.0, s_act[:, 384:512],
                        mult, mult)
                    nc.vector.scalar_tensor_tensor(
                        s_t2a[:, :], s_act[:, 128:256], 1.0, cold[:, 0:128],
                        mult, mult)
                    dve.drain()
                    nc.vector.scalar_tensor_tensor(
                        cnew[:, 0:128], s_t1a[:, :], 1.0, s_t2a[:, :],
                        mult, add).then_inc(s_cd, 1)
                    dve.wait_ge(s_tcs, 2 * t - 1)
                    nc.vector.scalar_tensor_tensor(
                        s_hraw[:, 0:128], s_act[:, 256:384], 1.0, s_tc[:, 0:128],
                        mult, mult).then_inc(s_hd, 1)
                    dve.wait_ge(s_tr, 2 * t - 1)
                    if t >= 3:
                        dve.wait_ge(s_send, 32 * (t - 2))
                    nc.vector.tensor_copy(stg[:, 0:64], p_tr0[:, :])\
                        .then_inc(s_hl, 1)
                    # ---- block r1: c, h, stage ----
                    dve.wait_ge(s_a1, t)
                    nc.vector.scalar_tensor_tensor(
                        s_t1b[:, :], s_act[:, 512:640], 1.0, s_act[:, 896:1024],
                        mult, mult)
                    nc.vector.scalar_tensor_tensor(
                        s_t2b[:, :], s_act[:, 640:768], 1.0, cold[:, 128:256],
                        mult, mult)
                    dve.drain()
                    nc.vector.scalar_tensor_tensor(
                        cnew[:, 128:256], s_t1b[:, :], 1.0, s_t2b[:, :],
                        mult, add).then_inc(s_cd, 1)
                    dve.wait_ge(s_tcs, 2 * t)
                    nc.vector.scalar_tensor_tensor(
                        s_hraw[:, 128:256], s_act[:, 768:896], 1.0,
                        s_tc[:, 128:256], mult, mult).then_inc(s_hd, 1)
                    dve.wait_ge(s_tr, 2 * t)
                    nc.vector.tensor_copy(stg[:, 64:128], p_tr1[:, :])\
                        .then_inc(s_hl, 1)
                # y copy
                dve.wait_ge(s_yv, t)
                if t >= 3:
                    dve.wait_ge(s_ydma, 16 * (t - 2))
                nc.vector.tensor_copy(s_y[:, ts((t - 1) % 2, 64)], p_y[:, :])\
                    .then_inc(s_ycp, 1)

        # ------------- gpsimd: h broadcast -------------
        # Descriptors for step t are PREPARED at the top of the step (no data
        # dependence) and only TRIGGERED once the staging copy lands, so the
        # ~0.8us Q7 desc-gen is off the h-exchange critical path. r0 rides
        # SWDGE queue 0, r1 queue 1, so the two transfers' engine processing
        # overlaps instead of serializing.
        @block.gpsimd
        def _(gpsimd):
            gpsimd.wait_ge(s_pre, 16 * N_PRE)
            pid = gpsimd.partition_id()
            rd = [(0, k) for k in range(NC)]    # all 8 peers incl. self
            for t in range(1, Tn):
                hn = s_h[t % 2]
                stg = s_stage[:, ts(t % 2, 128)]
                if t >= 3:
                    gpsimd.wait_ge(s_send, 32 * (t - 2))
                gpsimd.remote_dma_broadcast(
                    hn[:, ds(pid * 128, 64)], stg[:, 0:64],
                    remote_sem=s_hr0[t % 2], local_sem=s_send, rdests=rd)\
                    .then_inc(s_prep, 1)
                gpsimd.remote_dma_broadcast(
                    hn[:, ds(pid * 128 + 64, 64)], stg[:, 64:128],
                    remote_sem=s_hr1[t % 2], local_sem=s_send, rdests=rd)\
                    .then_inc(s_prep, 1)
                gpsimd.wait_ge(s_prep, 2 * t)
                if t == 1:
                    # fire both step-1 broadcasts in ONE doorbell batch so
                    # they deterministically share the one-time ~2.6ms RDMA
                    # cold window instead of sometimes serializing two of them
                    gpsimd.wait_ge(s_hl, 2)
                    gpsimd.trigger_dma(count=2)
                else:
                    gpsimd.wait_ge(s_hl, 2 * t - 1)
                    gpsimd.wait_ge(s_yv, t - 1)   # old-parity h reads done
                    gpsimd.trigger_dma(count=1)
                    gpsimd.wait_ge(s_hl, 2 * t)
                    gpsimd.trigger_dma(count=1)

    for c in reversed(ctx_list):
        c.__exit__(None, None, None)

    nc.compile()
    return nc


# ---------------------------------------------------------------------------
# host side
# ---------------------------------------------------------------------------

def _sigmoid(x):
    return 1.0 / (1.0 + np.exp(-x))


def prepare_inputs(inputs: dict):
    """Host-side fold + step 0; returns per-core in_maps."""
    h0 = np.asarray(inputs["h0"])[0].astype(np.float64)      # [B, H]
    c0 = np.asarray(inputs["c0"])[0].astype(np.float64)
    W_ih = np.asarray(inputs["W_ih"]).astype(np.float64)     # [4H, 512]
    W_hh = np.asarray(inputs["W_hh"]).astype(np.float64)     # [4H, H]
    b = (np.asarray(inputs["b_ih"]) + np.asarray(inputs["b_hh"])).astype(np.float64)
    W_out = np.asarray(inputs["W_out"]).astype(np.float64)   # [MEL, H]
    b_out = np.asarray(inputs["b_out"]).astype(np.float64)

    W_comb = W_hh + W_ih @ W_out                             # [4H, H]
    bp = b + W_ih @ b_out                                    # [4H]

    # host step 0 (x = 0)
    gates0 = h0 @ W_hh.T + b
    i0, f0, g0, o0 = np.split(gates0, 4, axis=1)
    c1 = _sigmoid(f0) * c0 + _sigmoid(i0) * np.tanh(g0)
    h1 = _sigmoid(o0) * np.tanh(c1)                          # h(0) [B, H]

    hT = np.ascontiguousarray(h1.T)                          # [H, B]
    h0t = hT.reshape(KCH, 128, B).transpose(1, 0, 2)\
        .reshape(128, KCH * B).astype(np.float16)

    in_maps = []
    GATE_ORDER = (0, 1, 3, 2)                                # [i|f|o|g]
    for s in range(NC):
        rows = np.concatenate(
            [np.arange(g * H + s * HS + r * 128, g * H + s * HS + r * 128 + 128)
             for r in range(2) for g in GATE_ORDER])         # [r0 512 | r1 512]
        WT = np.ascontiguousarray(W_comb[rows, :].T)         # [H, 1024]
        w = WT.reshape(KCH, 128, 1024).transpose(1, 0, 2)\
            .reshape(128, KCH * 1024).astype(np.float16)
        WyT = np.ascontiguousarray(W_out[s * 64:(s + 1) * 64, :].T)  # [H, 64]
        wy = WyT.reshape(KCH, 128, 64).transpose(1, 0, 2)\
            .reshape(128, KCH * 64).astype(np.float16)
        in_maps.append({
            "w": np.ascontiguousarray(w),
            "wy": np.ascontiguousarray(wy),
            "h0": np.ascontiguousarray(h0t),
            "c0": np.ascontiguousarray(
                c1[:, s * HS:(s + 1) * HS].astype(np.float32)),
            "bps": np.ascontiguousarray(bp[rows][None, :].astype(np.float16)),
            "bout": np.ascontiguousarray(
                b_out[s * 64:(s + 1) * 64][None, :].astype(np.float16)),
            "ones": np.ones((1, B), np.float16),
            "ident": np.eye(B, dtype=np.float32),
        })
    return in_maps


# ---------------------------------------------------------------------------
# harness entry point
# ---------------------------------------------------------------------------

def run(inputs, trace=False):
    nc = build_nc(T, debug=False)
    in_maps = prepare_inputs(inputs)
    from concourse import bass_utils
    return bass_utils.run_bass_kernel_spmd(
        nc, in_maps, core_ids=list(range(NC)), trace=trace)


def assemble_output(res):
    out = np.empty((B, T, MEL), np.float32)
    for s in range(NC):
        ys = res.results[s]["yout"]          # [T, B, 64]
        out[:, :, s * 64:(s + 1) * 64] = ys.transpose(1, 0, 2)
    return np.ascontiguousarray(out)


def kernel(**inputs):
    """Full-input/full-output entry. Distributes across 8 NeuronCores (TP over
    the 4H gate dim) internally; returns y [B, T, MEL] float32."""
    return assemble_output(run(inputs))
